# revision 27
# baseline (speedup 1.0000x reference)
"""Dot-product attention on 8 Trainium2 NeuronCores — v5 (head-sequential).

Full inputs [B=4, H=16, S=1024, D=64] fp32; 64 heads sharded 8 per core,
processed sequentially. A head's phase is a per-ki software pipeline with
THREE [128,1024] PSUM score slots (6 banks) + one [65,1024] output
accumulator (2 banks), so the MM1 -> exp -> slot-release ring has slack
and the engines, not the dependency ring, set the pace.

Per head h, per ki (8 k-tiles of 128):
  stage[k,q] = K^T d-major @ Q^T d-major   (fp16, two K=64 matmuls; the
                                            head's Q/K is duplicated in
                                            both PE row halves so the two
                                            q-half matmuls alternate row
                                            groups at 2 cols/cycle)
  E = ~exp(stage/8) fp16                   ScalarE true exp (6 of 8 ki) or
                                           VectorE dual-phase Schraudolph
                                           (2 of 8: A=rint(a*s+b) i16;
                                           B=A-512; E=f16(A)+f16(B), ~1.1%
                                           max err; common gain G matched
                                           on ScalarE via bias ln G)
  O^T+sums += [V | 1]^T @ E                (fp16, trails exp by 4 ki,
                                            q-halves share each LDW)
  drain: VectorE copy psum -> fp16 SBUF -> DMA out (unnormalized + sums)
Host: out[q,d] = (O^T[d,q] / sums[q])^T while gathering shards.

Toolchain notes (walrus 2026-05-04 + bass_rust skew):
 - walrus accepts at most ONE sync-wait per instruction; a JSON pass over
   the BIR hoists extra waits onto NoOps (same engine, in-order).
 - lower_dve crashes with ldw-opt enabled; keep it off.
"""

import json
from contextlib import ExitStack

import numpy as np

import concourse.bass as bass
import concourse.bass2jax as bass2jax
import concourse.mybir as mybir
import concourse.tile as tile
from concourse import bass_utils
from concourse.vector_clock import ScopedClock

F32 = mybir.dt.float32
F16 = mybir.dt.float16
I16 = mybir.dt.int16
Alu = mybir.AluOpType

N_CORES = 8
HEADS_PER_CORE = 8
S = 1024
D = 64
KT = S // 128  # 8 k-tiles per head
NPAIR = HEADS_PER_CORE // 2

SCALE = 0.125  # 1/sqrt(64)

# dual-phase Schraudolph constants (see calib3.py): A = rint(a1*s + b1)
# int16; E = fp16_bits(A) + fp16_bits(A-512) approximates G*exp(s*SCALE)
# with max rel err ~1.1% and unit gain (b1 phase chosen so G == 1).
EXP_A1 = 184.6649627685547
EXP_B1 = 14517.731933593746

# which of the 8 ki-stages per head phase use the VectorE fast-exp path
DVE_KIS = frozenset({2, 6})

_DRAIN_MAX_WAITS = 1


def _split_drain_and_barrier(self, tick_clock, wait_clock):
    nc = self.nc
    drain_inst = nc.sync.drain()
    wait_clock.add_sem_waits(
        drain_inst.ins, ScopedClock({None: tick_clock.global_clock})
    )
    si = drain_inst.ins.sync_info
    if si is not None and si.on_wait and len(si.on_wait) > _DRAIN_MAX_WAITS:
        waits = list(si.on_wait)
        updates = list(si.on_update or [])
        drain_inst.ins.sync_info = mybir.SyncInfo(
            on_wait=waits[:_DRAIN_MAX_WAITS], on_update=[]
        )
        rest = waits[_DRAIN_MAX_WAITS:]
        for i in range(0, len(rest), _DRAIN_MAX_WAITS):
            extra = nc.sync.drain()
            extra.ins.sync_info = mybir.SyncInfo(
                on_wait=rest[i : i + _DRAIN_MAX_WAITS],
                on_update=updates if i + _DRAIN_MAX_WAITS >= len(rest) else [],
            )
    nc.all_engine_barrier()
    assert self.sems is not None
    popped = nc._tile_sem_poison_stack.pop()
    assert popped is self._sem_poison
    nc.clear_and_free_semaphores(list(self.sems.allocated().values()))
    nc.all_engine_barrier()


def _prune_dominated_waits(j) -> None:
    """Drop provably-redundant sem waits: engines execute in order, and for
    inc-only (monotone) semaphores a wait (sem >= t) is implied by any
    EARLIER wait on the same engine queue with threshold >= t.  Fewer waits
    => fewer walrus-lowered standalone EVENT_SEMAPHORE/NoOp queue slots."""
    # semaphores ever updated by anything other than sem-inc are not
    # monotone (barrier dec etc.) — never prune waits on those
    bad = set()
    for f in j["functions"]:
        for b in f["blocks"]:
            for inst in b["instructions"]:
                si = inst.get("sync_info") or {}
                for u in si.get("on_update") or []:
                    if u.get("update_mode") != "sem-inc":
                        bad.add(u.get("id"))
    for f in j["functions"]:
        for b in f["blocks"]:
            seen = {}  # (engine, sem_id) -> max threshold already waited
            for inst in b["instructions"]:
                si = inst.get("sync_info")
                waits = (si or {}).get("on_wait") or []
                if not waits:
                    continue
                eng = inst["engine"]
                kept = {}  # sem_id -> max threshold this instruction
                for w in waits:
                    if (
                        w.get("sync_type") != "semaphore"
                        or w.get("wait_mode") != "sem-ge-imm"
                        or w.get("id") in bad
                    ):
                        kept[("raw", len(kept))] = w
                        continue
                    sid, t = w["id"], w["wait_value"]
                    if seen.get((eng, sid), -1) >= t:
                        continue  # dominated by an earlier wait on this queue
                    prev = kept.get(sid)
                    if prev is None or prev["wait_value"] < t:
                        kept[sid] = w
                for sid, w in kept.items():
                    if isinstance(sid, tuple):
                        continue
                    seen[(eng, sid)] = max(
                        seen.get((eng, sid), -1), w["wait_value"]
                    )
                si["on_wait"] = list(kept.values())


def _split_waits_in_bir(bir_json: bytes) -> bytes:
    """Hoist extra sync-waits onto NoOps inserted immediately before the
    owning instruction (same engine, in-order => semantics unchanged)."""
    j = json.loads(bir_json)
    _prune_dominated_waits(j)
    n = 0
    for f in j["functions"]:
        for b in f["blocks"]:
            out = []
            for inst in b["instructions"]:
                si = inst.get("sync_info")
                waits = (si or {}).get("on_wait") or []
                if len(waits) > 1:
                    for w in waits[:-1]:
                        out.append(
                            {
                                "debug": inst.get("debug", 0),
                                "engine": inst["engine"],
                                "ins": [],
                                "outs": [],
                                "name": f"{inst['name']}-wsplit{n}",
                                "opcode": "NoOp",
                                "sync_info": {"on_update": [], "on_wait": [w]},
                            }
                        )
                        n += 1
                    si["on_wait"] = [waits[-1]]
                out.append(inst)
            b["instructions"] = out
    return json.dumps(j).encode()


_orig_compile_bir_kernel = bass_utils.compile_bir_kernel


def _compile_bir_kernel_splitting(bir_json, tmpdir, neff_name="file.neff"):
    return _orig_compile_bir_kernel(_split_waits_in_bir(bir_json), tmpdir, neff_name)


ENABLE_LDW_OPT = False
_orig_run_command = bass_utils.run_command


def _run_command_ldw(argv, **kwargs):
    if ENABLE_LDW_OPT:
        argv = [
            a.replace("--enable-ldw-opt=false", "--enable-ldw-opt=true") for a in argv
        ]
    return _orig_run_command(argv, **kwargs)


def _install_patches():
    if not getattr(tile.TileContext, "_drain_split_installed", False):
        tile.TileContext._drain_and_barrier = _split_drain_and_barrier
        tile.TileContext._drain_split_installed = True
    if bass_utils.compile_bir_kernel is not _compile_bir_kernel_splitting:
        bass_utils.compile_bir_kernel = _compile_bir_kernel_splitting
        bass2jax.compile_bir_kernel = _compile_bir_kernel_splitting
        bass_utils.run_command = _run_command_ldw


def build_nc() -> bass.Bass:
    _install_patches()
    nc = bass.Bass(
        trn_type="TRN2", target_bir_lowering=False, debug=False, num_devices=N_CORES
    )
    # kq[h, 0:64, 0:1024] = Q^T head h ; [0:64, 1024:] = K^T head h
    # kq[h, 64:128, ...]   = the SAME data duplicated (row-group packing)
    kq = nc.dram_tensor(
        "kq", [HEADS_PER_CORE, 128, 2 * S], F16, kind="ExternalInput"
    ).ap()
    # vext[h, p, t, j]: V[h, 128*t + p, j] for j < 64, 1.0 at j == 64 (fp16)
    vext = nc.dram_tensor(
        "vext", [HEADS_PER_CORE, 128, KT, 65], F16, kind="ExternalInput"
    ).ap()
    # outu[h, 0:64, q] = unnormalized out^T ; outu[h, 64, q] = softmax sums
    outu = nc.dram_tensor(
        "outu", [HEADS_PER_CORE, 65, S], F16, kind="ExternalOutput"
    ).ap()

    with tile.TileContext(nc) as tc, ExitStack() as ctx:
        sb = ctx.enter_context(tc.tile_pool(name="sb", bufs=2))
        psS = ctx.enter_context(tc.tile_pool(name="psS", bufs=3, space="PSUM"))
        psO = ctx.enter_context(tc.tile_pool(name="psO", bufs=1, space="PSUM"))

        # pay the one-time ACT table load while the first DMAs stream
        singles = ctx.enter_context(tc.tile_pool(name="singles", bufs=1))
        warm_in = singles.tile([128, 16], F32, tag="warm_in")
        nc.vector.memset(warm_in, 0.0)
        warm_out = singles.tile([128, 16], F16, tag="warm_out")
        nc.scalar.activation(out=warm_out, in_=warm_in,
                             func=mybir.ActivationFunctionType.Exp, scale=1.0)

        kq_tiles = {}
        v_tiles = {}

        def prefetch_kq(h):
            kq_s = sb.tile([128, 2 * S], F16, tag="kq", name=f"kq_{h}")
            if h == 0:
                # first head: race the critical [Q | K ki0] region in on two
                # queues so MM1(0,0) starts ~0.8us earlier
                nc.sync.dma_start(kq_s[:, 0:576], kq[h][:, 0:576])
                nc.scalar.dma_start(kq_s[:, 576:1152], kq[h][:, 576:1152])
                nc.gpsimd.dma_start(kq_s[:, 1152:], kq[h][:, 1152:])
            else:
                nc.sync.dma_start(kq_s[:, : S + 128], kq[h][:, : S + 128])
                nc.sync.dma_start(kq_s[:, S + 128 :], kq[h][:, S + 128 :])
            kq_tiles[h] = kq_s

        def prefetch_v(h):
            v_s = sb.tile([128, KT, 65], F16, tag="v", name=f"v_{h}")
            nc.gpsimd.dma_start(v_s, vext[h])
            v_tiles[h] = v_s

        prefetch_kq(0)
        prefetch_v(0)

        heads = {}

        def ensure_head(h):
            heads[h] = {
                "kq_s": kq_tiles.pop(h),
                "v_s": v_tiles.pop(h),
                "e_s": sb.tile([128, KT, 2 * 512], F16, tag="e", name=f"e_{h}"),
                "o_ps": None,
            }

        def emit_stage(h, ki):
            hs = heads[h]
            kq_s = hs["kq_s"]
            st = psS.tile([128, 2 * 512], F32, tag="st", name=f"st_{h}_{ki}")
            for c in range(2):
                b0 = 64 * c
                nc.tensor.matmul(
                    st[:, c * 512 : (c + 1) * 512],
                    kq_s[b0 : b0 + 64, S + ki * 128 : S + (ki + 1) * 128],
                    kq_s[b0 : b0 + 64, c * 512 : (c + 1) * 512],
                    start=True,
                    stop=True,
                    tile_position=(b0, 0),
                )
            dst = hs["e_s"][:, ki]
            if ki in DVE_KIS:
                a_t = sb.tile([128, 1024], F16, tag="exp_a")
                nc.vector.tensor_scalar(
                    out=a_t.bitcast(I16), in0=st, scalar1=EXP_A1,
                    scalar2=EXP_B1, op0=Alu.mult, op1=Alu.add,
                )
                b_t = sb.tile([128, 1024], F16, tag="exp_b")
                nc.vector.tensor_scalar(
                    out=b_t.bitcast(I16), in0=a_t.bitcast(I16),
                    scalar1=-512, scalar2=None, op0=Alu.add,
                )
                nc.vector.tensor_add(dst, a_t, b_t)
            else:
                nc.scalar.activation(
                    out=dst, in_=st,
                    func=mybir.ActivationFunctionType.Exp, scale=SCALE,
                )

        def emit_mm2(h, kj):
            hs = heads[h]
            if kj == 0:
                hs["o_ps"] = psO.tile([65, 2 * 512], F32, tag="o",
                                      name=f"o_{h}")
            for c in range(2):
                nc.tensor.matmul(
                    hs["o_ps"][:, c * 512 : (c + 1) * 512],
                    hs["v_s"][:, kj, :],
                    hs["e_s"][:, kj, c * 512 : (c + 1) * 512],
                    start=(kj == 0),
                    stop=(kj == KT - 1),
                )

        # head 0's first two stages are emitted up front; each head's loop
        # then runs ki 2..7, and the NEXT head's ki 0/1 are peeled in before
        # this head's tail MM2 burst — so the exp engines never idle behind
        # the tail (the burst's queue position no longer gates MM1(h+1,0)).
        ensure_head(0)
        emit_stage(0, 0)
        emit_stage(0, 1)
        for h in range(HEADS_PER_CORE):
            for ki in range(2, KT):
                emit_stage(h, ki)
                if ki == 2 and h + 1 < HEADS_PER_CORE:
                    prefetch_v(h + 1)
                elif ki == 3 and h + 1 < HEADS_PER_CORE:
                    prefetch_kq(h + 1)
                if ki >= 4:
                    emit_mm2(h, ki - 4)
            if h + 1 < HEADS_PER_CORE:
                ensure_head(h + 1)
                emit_stage(h + 1, 0)
                emit_stage(h + 1, 1)
            for kj in range(KT - 4, KT):
                emit_mm2(h, kj)
            ou = sb.tile([65, 2 * 512], F16, tag="ou", name=f"ou_{h}")
            if h == HEADS_PER_CORE - 1:
                # split the final drain across both engines: shorter tail
                nc.scalar.copy(out=ou[:, :512], in_=heads[h]["o_ps"][:, :512])
                nc.vector.tensor_copy(ou[:, 512:], heads[h]["o_ps"][:, 512:])
            else:
                nc.vector.tensor_copy(ou, heads[h]["o_ps"])
            nc.sync.dma_start(outu[h], ou)

    return nc


def _shard_inputs(queries, keys, values):
    """Full [4,16,1024,64] fp32 -> per-core kq / vext (fp16)."""
    q = np.ascontiguousarray(queries, dtype=np.float32).reshape(64, S, D)
    k = np.ascontiguousarray(keys, dtype=np.float32).reshape(64, S, D)
    v = np.ascontiguousarray(values, dtype=np.float32).reshape(64, S, D)

    qT = q.transpose(0, 2, 1)  # [64, D, S]
    kT = k.transpose(0, 2, 1)

    kq = np.empty((64, 128, 2 * S), np.float16)
    kq[:, 0:64, 0:S] = qT
    kq[:, 0:64, S:] = kT
    kq[:, 64:128, 0:S] = qT
    kq[:, 64:128, S:] = kT

    vext = np.empty((64, 128, KT, 65), np.float16)
    vext[..., 64] = 1.0
    vext[..., :64] = v.reshape(64, KT, 128, D).transpose(0, 2, 1, 3)

    in_maps = []
    for c in range(N_CORES):
        in_maps.append(
            {
                "kq": np.ascontiguousarray(kq[c * 8 : (c + 1) * 8]),
                "vext": np.ascontiguousarray(vext[c * 8 : (c + 1) * 8]),
            }
        )
    return in_maps


_CACHE = {}


def _get_nc() -> bass.Bass:
    if "nc" not in _CACHE:
        _CACHE["nc"] = build_nc()
    return _CACHE["nc"]


def run(queries, keys, values, d_k, trace=False, trace_kwargs=None):
    assert int(d_k) == D
    nc = _get_nc()
    in_maps = _shard_inputs(queries, keys, values)
    res = bass_utils.run_bass_kernel_spmd(
        nc,
        in_maps,
        core_ids=list(range(N_CORES)),
        trace=trace,
        **(trace_kwargs or {}),
    )
    outu = np.stack([r["outu"] for r in res.results]).astype(np.float32)
    # [8 cores, 8 heads, 65, S] -> normalize + transpose
    outu = outu.reshape(64, 65, S)
    out = outu[:, 0:64, :] / outu[:, 64:65, :]  # [64, D, S]
    out = np.ascontiguousarray(out.transpose(0, 2, 1)).reshape(4, 16, S, D)
    return out.astype(np.float32), res


def kernel(queries, keys, values, d_k):
    out, _ = run(queries, keys, values, d_k, trace=False)
    return out



# revision 28
# speedup vs baseline: 1.0420x; 1.0420x over previous
"""Dot-product attention on 8 Trainium2 NeuronCores — v5 (head-sequential).

Full inputs [B=4, H=16, S=1024, D=64] fp32; 64 heads sharded 8 per core,
processed sequentially. A head's phase is a per-ki software pipeline with
THREE [128,1024] PSUM score slots (6 banks) + one [65,1024] output
accumulator (2 banks), so the MM1 -> exp -> slot-release ring has slack
and the engines, not the dependency ring, set the pace.

Per head h, per ki (8 k-tiles of 128):
  stage[k,q] = K^T d-major @ Q^T d-major   (fp16, two K=64 matmuls; the
                                            head's Q/K is duplicated in
                                            both PE row halves so the two
                                            q-half matmuls alternate row
                                            groups at 2 cols/cycle)
  E = ~exp(stage/8) fp16                   ScalarE true exp (6 of 8 ki) or
                                           VectorE dual-phase Schraudolph
                                           (2 of 8: A=rint(a*s+b) i16;
                                           B=A-512; E=f16(A)+f16(B), ~1.1%
                                           max err; common gain G matched
                                           on ScalarE via bias ln G)
  O^T+sums += [V | 1]^T @ E                (fp16, trails exp by 4 ki,
                                            q-halves share each LDW)
  drain: VectorE copy psum -> fp16 SBUF -> DMA out (unnormalized + sums)
Host: out[q,d] = (O^T[d,q] / sums[q])^T while gathering shards.

Toolchain notes (walrus 2026-05-04 + bass_rust skew):
 - walrus accepts at most ONE sync-wait per instruction; a JSON pass over
   the BIR hoists extra waits onto NoOps (same engine, in-order).
 - lower_dve crashes with ldw-opt enabled; keep it off.
"""

import json
from contextlib import ExitStack

import numpy as np

import concourse.bass as bass
import concourse.bass2jax as bass2jax
import concourse.mybir as mybir
import concourse.tile as tile
from concourse import bass_utils
from concourse.vector_clock import ScopedClock

F32 = mybir.dt.float32
F16 = mybir.dt.float16
I16 = mybir.dt.int16
Alu = mybir.AluOpType

N_CORES = 8
HEADS_PER_CORE = 8
S = 1024
D = 64
KT = S // 128  # 8 k-tiles per head
NPAIR = HEADS_PER_CORE // 2

SCALE = 0.125  # 1/sqrt(64)

# dual-phase Schraudolph constants (see calib3.py): A = rint(a1*s + b1)
# int16; E = fp16_bits(A) + fp16_bits(A-512) approximates G*exp(s*SCALE)
# with max rel err ~1.1% and unit gain (b1 phase chosen so G == 1).
EXP_A1 = 184.6649627685547
EXP_B1 = 14517.731933593746

# which of the 8 ki-stages per head phase use the VectorE fast-exp path
DVE_KIS = frozenset({2, 6})

_DRAIN_MAX_WAITS = 1


def _split_drain_and_barrier(self, tick_clock, wait_clock):
    nc = self.nc
    drain_inst = nc.sync.drain()
    wait_clock.add_sem_waits(
        drain_inst.ins, ScopedClock({None: tick_clock.global_clock})
    )
    si = drain_inst.ins.sync_info
    if si is not None and si.on_wait and len(si.on_wait) > _DRAIN_MAX_WAITS:
        waits = list(si.on_wait)
        updates = list(si.on_update or [])
        drain_inst.ins.sync_info = mybir.SyncInfo(
            on_wait=waits[:_DRAIN_MAX_WAITS], on_update=[]
        )
        rest = waits[_DRAIN_MAX_WAITS:]
        for i in range(0, len(rest), _DRAIN_MAX_WAITS):
            extra = nc.sync.drain()
            extra.ins.sync_info = mybir.SyncInfo(
                on_wait=rest[i : i + _DRAIN_MAX_WAITS],
                on_update=updates if i + _DRAIN_MAX_WAITS >= len(rest) else [],
            )
    nc.all_engine_barrier()
    assert self.sems is not None
    popped = nc._tile_sem_poison_stack.pop()
    assert popped is self._sem_poison
    nc.clear_and_free_semaphores(list(self.sems.allocated().values()))
    nc.all_engine_barrier()


def _split_waits_in_bir(bir_json: bytes) -> bytes:
    """Hoist extra sync-waits onto NoOps inserted immediately before the
    owning instruction (same engine, in-order => semantics unchanged)."""
    j = json.loads(bir_json)
    n = 0
    for f in j["functions"]:
        for b in f["blocks"]:
            out = []
            for inst in b["instructions"]:
                si = inst.get("sync_info")
                waits = (si or {}).get("on_wait") or []
                if len(waits) > 1:
                    for w in waits[:-1]:
                        out.append(
                            {
                                "debug": inst.get("debug", 0),
                                "engine": inst["engine"],
                                "ins": [],
                                "outs": [],
                                "name": f"{inst['name']}-wsplit{n}",
                                "opcode": "NoOp",
                                "sync_info": {"on_update": [], "on_wait": [w]},
                            }
                        )
                        n += 1
                    si["on_wait"] = [waits[-1]]
                out.append(inst)
            b["instructions"] = out
    return json.dumps(j).encode()


_orig_compile_bir_kernel = bass_utils.compile_bir_kernel


def _compile_bir_kernel_splitting(bir_json, tmpdir, neff_name="file.neff"):
    return _orig_compile_bir_kernel(_split_waits_in_bir(bir_json), tmpdir, neff_name)


ENABLE_LDW_OPT = False
_orig_run_command = bass_utils.run_command


def _run_command_ldw(argv, **kwargs):
    if ENABLE_LDW_OPT:
        argv = [
            a.replace("--enable-ldw-opt=false", "--enable-ldw-opt=true") for a in argv
        ]
    return _orig_run_command(argv, **kwargs)


def _install_patches():
    if not getattr(tile.TileContext, "_drain_split_installed", False):
        tile.TileContext._drain_and_barrier = _split_drain_and_barrier
        tile.TileContext._drain_split_installed = True
    if bass_utils.compile_bir_kernel is not _compile_bir_kernel_splitting:
        bass_utils.compile_bir_kernel = _compile_bir_kernel_splitting
        bass2jax.compile_bir_kernel = _compile_bir_kernel_splitting
        bass_utils.run_command = _run_command_ldw


def build_nc() -> bass.Bass:
    _install_patches()
    nc = bass.Bass(
        trn_type="TRN2", target_bir_lowering=False, debug=False, num_devices=N_CORES
    )
    # kq[h, 0:64, 0:1024] = Q^T head h ; [0:64, 1024:] = K^T head h
    # kq[h, 64:128, ...]   = the SAME data duplicated (row-group packing)
    kq = nc.dram_tensor(
        "kq", [HEADS_PER_CORE, 128, 2 * S], F16, kind="ExternalInput"
    ).ap()
    # vext[h, p, t, j]: V[h, 128*t + p, j] for j < 64, 1.0 at j == 64 (fp16)
    vext = nc.dram_tensor(
        "vext", [HEADS_PER_CORE, 128, KT, 65], F16, kind="ExternalInput"
    ).ap()
    # outu[h, 0:64, q] = unnormalized out^T ; outu[h, 64, q] = softmax sums
    outu = nc.dram_tensor(
        "outu", [HEADS_PER_CORE, 65, S], F16, kind="ExternalOutput"
    ).ap()

    with tile.TileContext(nc) as tc, ExitStack() as ctx:
        sb = ctx.enter_context(tc.tile_pool(name="sb", bufs=2))
        psS = ctx.enter_context(tc.tile_pool(name="psS", bufs=3, space="PSUM"))
        psO = ctx.enter_context(tc.tile_pool(name="psO", bufs=1, space="PSUM"))

        # pay the one-time ACT table load while the first DMAs stream
        singles = ctx.enter_context(tc.tile_pool(name="singles", bufs=1))
        warm_in = singles.tile([128, 16], F32, tag="warm_in")
        nc.vector.memset(warm_in, 0.0)
        warm_out = singles.tile([128, 16], F16, tag="warm_out")
        nc.scalar.activation(out=warm_out, in_=warm_in,
                             func=mybir.ActivationFunctionType.Exp, scale=1.0)

        kq_tiles = {}
        v_tiles = {}

        def prefetch_kq(h):
            kq_s = sb.tile([128, 2 * S], F16, tag="kq", name=f"kq_{h}")
            nc.sync.dma_start(kq_s[:, : S + 128], kq[h][:, : S + 128])
            nc.sync.dma_start(kq_s[:, S + 128 :], kq[h][:, S + 128 :])
            kq_tiles[h] = kq_s

        def prefetch_v(h):
            v_s = sb.tile([128, KT, 65], F16, tag="v", name=f"v_{h}")
            nc.gpsimd.dma_start(v_s, vext[h])
            v_tiles[h] = v_s

        prefetch_kq(0)
        prefetch_v(0)

        heads = {}

        def ensure_head(h):
            heads[h] = {
                "kq_s": kq_tiles.pop(h),
                "v_s": v_tiles.pop(h),
                "e_s": sb.tile([128, KT, 2 * 512], F16, tag="e", name=f"e_{h}"),
                "o_ps": None,
            }

        def emit_stage(h, ki):
            hs = heads[h]
            kq_s = hs["kq_s"]
            st = psS.tile([128, 2 * 512], F32, tag="st", name=f"st_{h}_{ki}")
            for c in range(2):
                b0 = 64 * c
                nc.tensor.matmul(
                    st[:, c * 512 : (c + 1) * 512],
                    kq_s[b0 : b0 + 64, S + ki * 128 : S + (ki + 1) * 128],
                    kq_s[b0 : b0 + 64, c * 512 : (c + 1) * 512],
                    start=True,
                    stop=True,
                    tile_position=(b0, 0),
                )
            dst = hs["e_s"][:, ki]
            if ki in DVE_KIS:
                a_t = sb.tile([128, 1024], F16, tag="exp_a")
                nc.vector.tensor_scalar(
                    out=a_t.bitcast(I16), in0=st, scalar1=EXP_A1,
                    scalar2=EXP_B1, op0=Alu.mult, op1=Alu.add,
                )
                b_t = sb.tile([128, 1024], F16, tag="exp_b")
                nc.vector.tensor_scalar(
                    out=b_t.bitcast(I16), in0=a_t.bitcast(I16),
                    scalar1=-512, scalar2=None, op0=Alu.add,
                )
                nc.vector.tensor_add(dst, a_t, b_t)
            else:
                nc.scalar.activation(
                    out=dst, in_=st,
                    func=mybir.ActivationFunctionType.Exp, scale=SCALE,
                )

        def emit_mm2(h, kj):
            hs = heads[h]
            if kj == 0:
                hs["o_ps"] = psO.tile([65, 2 * 512], F32, tag="o",
                                      name=f"o_{h}")
            for c in range(2):
                nc.tensor.matmul(
                    hs["o_ps"][:, c * 512 : (c + 1) * 512],
                    hs["v_s"][:, kj, :],
                    hs["e_s"][:, kj, c * 512 : (c + 1) * 512],
                    start=(kj == 0),
                    stop=(kj == KT - 1),
                )

        # head 0's first two stages are emitted up front; each head's loop
        # then runs ki 2..7, and the NEXT head's ki 0/1 are peeled in before
        # this head's tail MM2 burst — so the exp engines never idle behind
        # the tail (the burst's queue position no longer gates MM1(h+1,0)).
        ensure_head(0)
        emit_stage(0, 0)
        emit_stage(0, 1)
        for h in range(HEADS_PER_CORE):
            for ki in range(2, KT):
                emit_stage(h, ki)
                if ki == 2 and h + 1 < HEADS_PER_CORE:
                    prefetch_v(h + 1)
                elif ki == 3 and h + 1 < HEADS_PER_CORE:
                    prefetch_kq(h + 1)
                if ki >= 4:
                    emit_mm2(h, ki - 4)
            if h + 1 < HEADS_PER_CORE:
                ensure_head(h + 1)
                emit_stage(h + 1, 0)
                emit_stage(h + 1, 1)
            for kj in range(KT - 4, KT):
                emit_mm2(h, kj)
            ou = sb.tile([65, 2 * 512], F16, tag="ou", name=f"ou_{h}")
            nc.vector.tensor_copy(ou, heads[h]["o_ps"])
            nc.sync.dma_start(outu[h], ou)

    return nc


def _shard_inputs(queries, keys, values):
    """Full [4,16,1024,64] fp32 -> per-core kq / vext (fp16)."""
    q = np.ascontiguousarray(queries, dtype=np.float32).reshape(64, S, D)
    k = np.ascontiguousarray(keys, dtype=np.float32).reshape(64, S, D)
    v = np.ascontiguousarray(values, dtype=np.float32).reshape(64, S, D)

    qT = q.transpose(0, 2, 1)  # [64, D, S]
    kT = k.transpose(0, 2, 1)

    kq = np.empty((64, 128, 2 * S), np.float16)
    kq[:, 0:64, 0:S] = qT
    kq[:, 0:64, S:] = kT
    kq[:, 64:128, 0:S] = qT
    kq[:, 64:128, S:] = kT

    vext = np.empty((64, 128, KT, 65), np.float16)
    vext[..., 64] = 1.0
    vext[..., :64] = v.reshape(64, KT, 128, D).transpose(0, 2, 1, 3)

    in_maps = []
    for c in range(N_CORES):
        in_maps.append(
            {
                "kq": np.ascontiguousarray(kq[c * 8 : (c + 1) * 8]),
                "vext": np.ascontiguousarray(vext[c * 8 : (c + 1) * 8]),
            }
        )
    return in_maps


_CACHE = {}


def _get_nc() -> bass.Bass:
    if "nc" not in _CACHE:
        _CACHE["nc"] = build_nc()
    return _CACHE["nc"]


def run(queries, keys, values, d_k, trace=False, trace_kwargs=None):
    assert int(d_k) == D
    nc = _get_nc()
    in_maps = _shard_inputs(queries, keys, values)
    res = bass_utils.run_bass_kernel_spmd(
        nc,
        in_maps,
        core_ids=list(range(N_CORES)),
        trace=trace,
        **(trace_kwargs or {}),
    )
    outu = np.stack([r["outu"] for r in res.results]).astype(np.float32)
    # [8 cores, 8 heads, 65, S] -> normalize + transpose
    outu = outu.reshape(64, 65, S)
    out = outu[:, 0:64, :] / outu[:, 64:65, :]  # [64, D, S]
    out = np.ascontiguousarray(out.transpose(0, 2, 1)).reshape(4, 16, S, D)
    return out.astype(np.float32), res


def kernel(queries, keys, values, d_k):
    out, _ = run(queries, keys, values, d_k, trace=False)
    return out

